# revision 11
# baseline (speedup 1.0000x reference)
"""AttentionBlock (GroupNorm + single-head self-attention + proj + residual)
for Trainium2, 8 NeuronCores, data-parallel over (batch, token-half).

Shapes (hardcoded): x [4, 256, 64, 64] fp32, weights [256, 256] fp32.
Each core handles one (batch b, token-half h): 2048 query tokens against the
full 4096 keys/values of its batch, entirely in SBUF.

v2: fp8(e4m3) DoubleRow matmuls for QKV production, scores and PV
(K=256 contraction in one instruction at 0.5 cyc/row); GroupNorm applied as
an explicit normalized image x8 = seff*x + beff cast to fp8, so the conv
weights are host-precast fp8 constants (scaled by 8 to stay in fp8's normal
range; compensated exactly by the exp argument scale and a 1/8 output
scale). Scores are exp'd in [128,1024] two-bank PSUM tiles; exp runs on ACT
(exact spline) for most tiles and optionally on DVE via a Schraudolph
int16-bitcast-fp16 + fp16->fp8 convert for load balancing. The softmax
denominator comes from a ones-column appended to V; normalization, the
attn transpose (PE, fp16), fp16 projection, +bias (ACT, with the 1/8
compensation), residual add and DMA-out complete the pipeline.

Accuracy: fp8 quantization of x8/w/k/q/u/v perturbs softmax weights by a few
percent, but the output is residual-dominated (||proj||/||x|| ~ 2.6%), so
end-to-end relative error stays ~1-3e-3, well under the 2e-2 gate.
"""

import sys

try:
    import concourse.bass as bass  # noqa: F401
except ImportError:
    sys.path.insert(0, "/opt/trn_rl_repo")

import numpy as np
import ml_dtypes

import concourse.bass as bass
import concourse.mybir as mybir
import concourse.tile as tile
from concourse.bass import ts
from concourse.bass_utils import run_bass_kernel_spmd
from concourse.masks import make_identity

FP = mybir.dt.float32
F16 = mybir.dt.float16
F8 = mybir.dt.float8e4
I16 = mybir.dt.int16
AF = mybir.ActivationFunctionType
ALU = mybir.AluOpType
AX = mybir.AxisListType
DR = mybir.MatmulPerfMode.DoubleRow

P = 128
C = 256
HW = 4096
HALF = 2048
NCH = 2          # channel chunks of 128
NJT = 32         # 128-wide key tiles
NJP = 16         # key-tile pairs (256 keys each)
NIG = 4          # query i-groups of 512
NCHUNK = 8       # 512-wide token chunks of the full image
GROUPS = 32
GSIZE = C // GROUPS          # 8 channels per group
NELEM = GSIZE * HW           # 32768 elements per group
EPS = 1e-6
WS = 8.0                     # host weight prescale (wq,wk,wv *= 8)
SCALE = float(C) ** -0.5     # 0.0625 softmax scale
EXPB = -3.0                  # global exp bias: u = exp(s*SCALE + EXPB); real score tails reach ~7.5 sigma, keep exp(max)+margin under fp8's 240
# PSUM holds 64*s (8x on q and k), so the activation scale is SCALE/64
PSC = SCALE / (WS * WS)
# Schraudolph fp16 constants: bits16 = round(a16*s_psum + b16), bitcast fp16
A16 = 1024.0 / np.log(2.0) * PSC
B16 = 1024.0 * (15.0 + EXPB / np.log(2.0)) - 60.0


def _split_waits(nc, max_waits=1):
    """The pinned walrus rejects >1 sync-wait on ctrl instructions; hoist
    excess waits onto preceding NoOps on the same engine (same instruction
    stream, so ordering is preserved)."""
    ctr = 0
    for bb in nc.m.functions[0].blocks:
        out = []
        changed = False
        for inst in bb.instructions:
            si = getattr(inst, "sync_info", None)
            waits = list(si.on_wait) if (si and si.on_wait) else []
            if len(waits) > max_waits:
                changed = True
                head, rest = waits[:-max_waits], waits[-max_waits:]
                for k in range(0, len(head), max_waits):
                    ctr += 1
                    nop = mybir.InstNoOp(name=f"I-wsplit-{ctr}", ins=[], outs=[])
                    nop.engine = inst.engine
                    nop.sync_info = mybir.SyncInfo(
                        on_wait=head[k : k + max_waits], on_update=[]
                    )
                    out.append(nop)
                inst.sync_info = mybir.SyncInfo(
                    on_wait=rest, on_update=list(si.on_update or [])
                )
            out.append(inst)
        if changed:
            bb.instructions = out


def build_nc(split=True, reps=1, n_dve_exp=0, gp_resid=True,
             dr_s=True, dr_pv=True, dr_prod=True):
    """n_dve_exp: how many of the 64 (g,jp) exp tiles run on DVE
    (Schraudolph) instead of ACT. gp_resid: residual add on GPSIMD."""
    nc = bass.Bass()
    xf = nc.dram_tensor("xf", [C, HW], FP, kind="ExternalInput")
    w8q = nc.dram_tensor("w8q", [P, NCH, C], F8, kind="ExternalInput")
    w8k = nc.dram_tensor("w8k", [P, NCH, C], F8, kind="ExternalInput")
    w8v = nc.dram_tensor("w8v", [P, NCH, C], F8, kind="ExternalInput")
    wp16 = nc.dram_tensor("wp16", [P, NCH, C], F16, kind="ExternalInput")
    bq8 = nc.dram_tensor("bq8", [P, NCH], FP, kind="ExternalInput")
    bp2 = nc.dram_tensor("bp2", [P, NCH], FP, kind="ExternalInput")
    gns = nc.dram_tensor("gns", [P, NCH], FP, kind="ExternalInput")
    gnb = nc.dram_tensor("gnb", [P, NCH], FP, kind="ExternalInput")
    gsel = nc.dram_tensor("gsel", [P, P], FP, kind="ExternalInput")
    y = nc.dram_tensor("y", [C, HALF], FP, kind="ExternalOutput")

    # static exp-engine schedule: spread DVE tiles evenly over the 64 slots
    dve_slot = [False] * (NIG * NJP)
    if n_dve_exp > 0:
        step = (NIG * NJP) / float(n_dve_exp)
        for i in range(n_dve_exp):
            dve_slot[min(63, int(i * step + step / 2))] = True

    with tile.TileContext(nc) as tc:
        with (
            tc.tile_pool(name="wts", bufs=1) as wts,
            tc.tile_pool(name="big", bufs=1) as big,
            tc.tile_pool(name="upool", bufs=6) as upool,
            tc.tile_pool(name="small", bufs=3) as small,
            tc.tile_pool(name="stats", bufs=1) as stats,
            tc.tile_pool(name="outp", bufs=3) as outp,
            tc.tile_pool(name="psS", bufs=2, space="PSUM") as psS,
            tc.tile_pool(name="psAcc", bufs=4, space="PSUM") as psAcc,
        ):
            # ---------------- input image first (critical path) ----------------
            xf_sb = big.tile([P, NCH, HW], FP, tag="xf")
            dma_engines = [nc.sync, nc.gpsimd, nc.scalar]
            for o in range(NCH):
                for t8 in range(NCHUNK):
                    eng = dma_engines[(o * NCHUNK + t8) % len(dma_engines)]
                    eng.dma_start(
                        out=xf_sb[:, o, ts(t8, 512)],
                        in_=xf[o * P : (o + 1) * P, ts(t8, 512)],
                    )

            # ---------------- constants ----------------
            w_sb = {}
            for name, dram, dt_ in (
                ("w8q", w8q, F8), ("w8k", w8k, F8), ("w8v", w8v, F8),
                ("wp16", wp16, F16),
            ):
                t = wts.tile([P, NCH, C], dt_, tag=f"w_{name}")
                nc.sync.dma_start(out=t, in_=dram[:, :, :])
                w_sb[name] = t
            gsel_sb = wts.tile([P, P], FP, tag="gsel")
            nc.sync.dma_start(out=gsel_sb, in_=gsel[:, :])
            bq_sb = wts.tile([P, NCH], FP, tag="bq")
            nc.sync.dma_start(out=bq_sb, in_=bq8[:, :])
            bp_sb = wts.tile([P, NCH], FP, tag="bp")
            nc.sync.dma_start(out=bp_sb, in_=bp2[:, :])
            gns_sb = wts.tile([P, NCH], FP, tag="gns")
            nc.sync.dma_start(out=gns_sb, in_=gns[:, :])
            gnb_sb = wts.tile([P, NCH], FP, tag="gnb")
            nc.sync.dma_start(out=gnb_sb, in_=gnb[:, :])
            ident_fp = wts.tile([P, P], FP, tag="ident_fp")
            make_identity(nc, ident_fp)
            ident = wts.tile([P, P], F16, tag="ident")
            nc.vector.tensor_copy(ident, ident_fp)
            expb_sb = wts.tile([P, 1], FP, tag="expb")
            nc.vector.memset(expb_sb, EXPB)

            for _rep in range(reps):
                # ---------------- phase 1a: GroupNorm statistics ----------------
                sum_cols = stats.tile([P, NCH, NCHUNK], FP, tag="sumc")
                sq_cols = stats.tile([P, NCH, NCHUNK], FP, tag="sqc")
                for t8 in range(NCHUNK):
                    for o in range(NCH):
                        nc.vector.tensor_reduce(
                            out=sum_cols[:, o, t8 : t8 + 1],
                            in_=xf_sb[:, o, ts(t8, 512)],
                            axis=AX.X, op=ALU.add,
                        )
                        sq_scr = small.tile([P, 512], FP, tag="sqscr")
                        nc.scalar.activation(
                            out=sq_scr, in_=xf_sb[:, o, ts(t8, 512)],
                            func=AF.Square,
                            accum_out=sq_cols[:, o, t8 : t8 + 1],
                        )

                seff = stats.tile([P, NCH], FP, tag="seff")
                beff = stats.tile([P, NCH], FP, tag="beff")
                eps_sb = stats.tile([P, 1], FP, tag="eps")
                nc.vector.memset(eps_sb, EPS)
                for o in range(NCH):
                    part = stats.tile([P, 2], FP, tag=f"part{o}")
                    nc.vector.tensor_reduce(
                        out=part[:, 0:1], in_=sum_cols[:, o], axis=AX.X, op=ALU.add
                    )
                    nc.vector.tensor_reduce(
                        out=part[:, 1:2], in_=sq_cols[:, o], axis=AX.X, op=ALU.add
                    )
                    gps = psAcc.tile([P, 258], FP, tag="acc", name="gps")[:, :2]
                    nc.tensor.matmul(gps, lhsT=gsel_sb, rhs=part, start=True, stop=True)
                    mean = stats.tile([P, 1], FP, tag=f"mean{o}")
                    nc.vector.tensor_scalar_mul(mean, gps[:, 0:1], 1.0 / NELEM)
                    ex2 = stats.tile([P, 1], FP, tag=f"ex2{o}")
                    nc.vector.tensor_scalar_mul(ex2, gps[:, 1:2], 1.0 / NELEM)
                    msq = stats.tile([P, 1], FP, tag=f"msq{o}")
                    nc.vector.tensor_mul(msq, mean, mean)
                    var = stats.tile([P, 1], FP, tag=f"var{o}")
                    nc.vector.tensor_tensor(var, ex2, msq, ALU.subtract)
                    # rstd = exp(-0.5 * ln(var + eps)) — stays in the exp table set
                    lnv = stats.tile([P, 1], FP, tag=f"lnv{o}")
                    nc.scalar.activation(out=lnv, in_=var, func=AF.Ln, bias=eps_sb)
                    rstd = stats.tile([P, 1], FP, tag=f"rstd{o}")
                    nc.scalar.activation(out=rstd, in_=lnv, func=AF.Exp, scale=-0.5)
                    nc.vector.tensor_mul(seff[:, o : o + 1], gns_sb[:, o : o + 1], rstd)
                    tmp = stats.tile([P, 1], FP, tag=f"tmp{o}")
                    nc.vector.tensor_mul(tmp, mean, seff[:, o : o + 1])
                    nc.vector.tensor_tensor(
                        beff[:, o : o + 1], gnb_sb[:, o : o + 1], tmp, ALU.subtract
                    )

                # ---------------- phase 1b: normalized image in fp8 ----------------
                x8 = big.tile([P, NCH, HW], F8, tag="x8")
                for o in range(NCH):
                    for t8 in range(NCHUNK):
                        nc.vector.tensor_scalar(
                            out=x8[:, o, ts(t8, 512)],
                            in0=xf_sb[:, o, ts(t8, 512)],
                            scalar1=seff[:, o : o + 1],
                            scalar2=beff[:, o : o + 1],
                            op0=ALU.mult, op1=ALU.add,
                        )

                # ---------------- phase 1c: K, V, Q (fp8 DoubleRow) ----------------
                k8 = big.tile([P, NCH, HW], F8, tag="k8")
                v8 = big.tile([P, NJT, C + 16], F8, tag="v8")  # 272-stride: DR subtile step must be 16B-aligned
                nc.vector.memset(v8[:, :, C : C + 16], 0.0)
                nc.vector.memset(v8[:, :, C : C + 1], 1.0)
                # K: pairs of 512-token chunks -> one 2-bank PSUM -> one copy
                for oo in range(NCH):
                    for t4 in range(4):
                        ps = psS.tile([P, 1024], FP, tag="psS", name="psk")
                        for h in range(2):
                            if dr_prod:
                                nc.tensor.matmul(
                                    ps[:, h * 512 : (h + 1) * 512],
                                    lhsT=w_sb["w8k"][:, :, oo * P : (oo + 1) * P],
                                    rhs=x8[:, :, ts(2 * t4 + h, 512)],
                                    start=True, stop=True, perf_mode=DR,
                                )
                            else:
                                for ci in range(NCH):
                                    nc.tensor.matmul(
                                        ps[:, h * 512 : (h + 1) * 512],
                                        lhsT=w_sb["w8k"][:, ci, oo * P : (oo + 1) * P],
                                        rhs=x8[:, ci, ts(2 * t4 + h, 512)],
                                        start=(ci == 0), stop=(ci == NCH - 1),
                                    )
                        nc.vector.tensor_copy(
                            out=k8[:, oo, ts(t4, 1024)], in_=ps
                        )
                # V: 4 j-tiles (4 x [P,256]) per 2-bank PSUM -> one copy
                for q4 in range(NCHUNK):
                    ps = psS.tile([P, 4, 256], FP, tag="psS", name="psv")
                    for h in range(4):
                        j = q4 * 4 + h
                        if dr_prod:
                            nc.tensor.matmul(
                                ps[:, h],
                                lhsT=x8[:, :, j * P : (j + 1) * P],
                                rhs=w_sb["w8v"][:, :, :],
                                start=True, stop=True, perf_mode=DR,
                            )
                        else:
                            for ci in range(NCH):
                                nc.tensor.matmul(
                                    ps[:, h],
                                    lhsT=x8[:, ci, j * P : (j + 1) * P],
                                    rhs=w_sb["w8v"][:, ci, :],
                                    start=(ci == 0), stop=(ci == NCH - 1),
                                )
                    nc.scalar.copy(out=v8[:, ts(q4, 4), 0:C], in_=ps)
                # Q: pairs of 512-query chunks, bias fused into evacuation
                q8t = big.tile([P, NCH, HALF], F8, tag="q8")
                for oo in range(NCH):
                    for t2 in range(2):
                        ps = psS.tile([P, 1024], FP, tag="psS", name="psq")
                        for h in range(2):
                            if dr_prod:
                                nc.tensor.matmul(
                                    ps[:, h * 512 : (h + 1) * 512],
                                    lhsT=w_sb["w8q"][:, :, oo * P : (oo + 1) * P],
                                    rhs=x8[:, :, ts(2 * t2 + h, 512)],
                                    start=True, stop=True, perf_mode=DR,
                                )
                            else:
                                for ci in range(NCH):
                                    nc.tensor.matmul(
                                        ps[:, h * 512 : (h + 1) * 512],
                                        lhsT=w_sb["w8q"][:, ci, oo * P : (oo + 1) * P],
                                        rhs=x8[:, ci, ts(2 * t2 + h, 512)],
                                        start=(ci == 0), stop=(ci == NCH - 1),
                                    )
                        nc.vector.tensor_scalar_add(
                            q8t[:, oo, ts(t2, 1024)], ps, bq_sb[:, oo : oo + 1]
                        )

                # ---------------- phase 2: attention ----------------
                att16 = big.tile([P, NCH, HALF], F16, tag="att")
                slot = 0
                for g in range(NIG):
                    acc = [
                        psAcc.tile([P, C + 2], FP, tag="acc", name=f"acc{g}_{t}")
                        for t in range(4)
                    ]
                    for jp in range(NJP):
                        ps = psS.tile([P, 2, 512], FP, tag="psS", name="pss")
                        for h in range(2):
                            j = 2 * jp + h
                            if dr_s:
                                nc.tensor.matmul(
                                    ps[:, h],
                                    lhsT=k8[:, :, j * P : (j + 1) * P],
                                    rhs=q8t[:, :, ts(g, 512)],
                                    start=True, stop=True, perf_mode=DR,
                                )
                            else:
                                for ci in range(NCH):
                                    nc.tensor.matmul(
                                        ps[:, h],
                                        lhsT=k8[:, ci, j * P : (j + 1) * P],
                                        rhs=q8t[:, ci, ts(g, 512)],
                                        start=(ci == 0), stop=(ci == NCH - 1),
                                    )
                        u2 = upool.tile([P, 2, 512], F8, tag="u")
                        if dve_slot[slot]:
                            # Schraudolph: bits16 = a*s + b, bitcast fp16, ->fp8
                            u16 = upool.tile([P, 2, 512], I16, tag="u16")
                            nc.vector.tensor_scalar(
                                out=u16, in0=ps, scalar1=float(A16),
                                scalar2=float(B16), op0=ALU.mult, op1=ALU.add,
                            )
                            nc.vector.tensor_copy(u2, u16.bitcast(F16))
                        else:
                            nc.scalar.activation(
                                out=u2, in_=ps, func=AF.Exp, scale=PSC,
                                bias=expb_sb,
                            )
                        slot += 1
                        for t in range(4):
                            if dr_pv:
                                nc.tensor.matmul(
                                    acc[t],
                                    lhsT=u2[:, :, t * P : (t + 1) * P],
                                    rhs=v8[:, 2 * jp : 2 * jp + 2, 0 : C + 2],
                                    start=(jp == 0), stop=(jp == NJP - 1),
                                    perf_mode=DR,
                                )
                            else:
                                for h in range(2):
                                    nc.tensor.matmul(
                                        acc[t],
                                        lhsT=u2[:, h, t * P : (t + 1) * P],
                                        rhs=v8[:, 2 * jp + h, 0 : C + 2],
                                        start=(jp == 0 and h == 0),
                                        stop=(jp == NJP - 1 and h == 1),
                                    )
                    for t in range(4):
                        rs = small.tile([P, 1], FP, tag="rs")
                        nc.vector.reciprocal(rs, acc[t][:, C : C + 1])
                        asb = small.tile([P, C], F16, tag="asb")
                        nc.vector.tensor_scalar_mul(asb, acc[t][:, 0:C], rs)
                        tps = psS.tile([P, 2, P], F16, tag="psS", name="tps")
                        for o in range(NCH):
                            nc.tensor.transpose(
                                tps[:, o], asb[:, o * P : (o + 1) * P], ident
                            )
                        col = g * 512 + t * P
                        nc.scalar.copy(
                            out=att16[:, :, col : col + P], in_=tps
                        )
                    # projection + residual + DMA out
                    for oo in range(NCH):
                        col = g * 512
                        ps = psS.tile([P, 1024], FP, tag="psS", name="psp")[:, :512]
                        for oi in range(NCH):
                            nc.tensor.matmul(
                                ps,
                                lhsT=w_sb["wp16"][:, oi, oo * P : (oo + 1) * P],
                                rhs=att16[:, oi, col : col + 512],
                                start=(oi == 0), stop=(oi == NCH - 1),
                            )
                        ot = outp.tile([P, 512], FP, tag="out", name="ot")
                        nc.scalar.activation(
                            out=ot, in_=ps, func=AF.Identity,
                            bias=bp_sb[:, oo : oo + 1], scale=1.0 / WS,
                        )
                        if gp_resid:
                            nc.gpsimd.tensor_tensor(
                                ot, ot, xf_sb[:, oo, ts(g, 512)], ALU.add
                            )
                        else:
                            nc.vector.tensor_add(
                                ot, ot, xf_sb[:, oo, ts(g, 512)]
                            )
                        nc.sync.dma_start(
                            out=y[oo * P : (oo + 1) * P, col : col + 512],
                            in_=ot,
                        )

    if split:
        _split_waits(nc)
    return nc


_NC_CACHE = None


def _get_nc():
    global _NC_CACHE
    if _NC_CACHE is None:
        _NC_CACHE = build_nc()
    return _NC_CACHE


def make_in_maps(x, gn_scale, gn_bias, wq, bq, wk, bk, wv, bv, wp, bp):
    B = x.shape[0]
    f32 = np.float32
    f8 = ml_dtypes.float8_e4m3

    def col2(v):  # [256] -> [128, 2], column o = channels o*128..o*128+127
        return np.ascontiguousarray(np.asarray(v, f32).reshape(NCH, P).T)

    def w3d(w, dt, scale=1.0):
        # [C(out), C(in)] -> [P, NCH(ci), C(out)]: [p, ci, co] = w[co, ci*P+p]
        wt = np.asarray(w, f32).T * scale            # [ci_full, co]
        wt = np.clip(wt, -240.0, 240.0)
        return np.ascontiguousarray(
            wt.reshape(NCH, P, C).transpose(1, 0, 2)
        ).astype(dt)

    w8q_h = w3d(wq, f8, WS)
    w8k_h = w3d(wk, f8, WS)
    w8v_h = w3d(wv, f8, WS)
    wp16_h = w3d(wp, np.float16, 1.0)
    bp2 = col2(np.asarray(wp, f32) @ np.asarray(bv, f32) + np.asarray(bp, f32))
    bq8 = col2(np.asarray(bq, f32) * WS)
    gns = col2(gn_scale)
    gnb = col2(gn_bias)
    gsel = np.kron(np.eye(P // GSIZE, dtype=f32), np.ones((GSIZE, GSIZE), f32))

    xr = np.asarray(x, f32).reshape(B, C, HW)
    in_maps = []
    for core in range(8):
        b, h = core // 2, core % 2
        # rotate so this core's query half sits at columns 0..HALF-1
        # (attention/groupnorm are token-order invariant)
        xfb = np.ascontiguousarray(np.roll(xr[b], -h * HALF, axis=1))
        in_maps.append(
            {
                "xf": xfb,
                "w8q": w8q_h, "w8k": w8k_h, "w8v": w8v_h, "wp16": wp16_h,
                "bq8": bq8, "bp2": bp2, "gns": gns, "gnb": gnb, "gsel": gsel,
            }
        )
    return in_maps


def assemble_out(results, B=4):
    out = np.empty((B, C, HW), np.float32)
    for core in range(8):
        b, h = core // 2, core % 2
        out[b, :, h * HALF : (h + 1) * HALF] = results[core]["y"]
    return out.reshape(B, C, 64, 64)


def kernel(**inputs):
    in_maps = make_in_maps(**inputs)
    nc = _get_nc()
    res = run_bass_kernel_spmd(nc, in_maps, list(range(8)))
    return assemble_out(res.results, B=inputs["x"].shape[0])


if __name__ == "__main__":
    rng = np.random.default_rng(0)
    ins = {
        "x": rng.standard_normal((4, C, 64, 64)).astype(np.float32),
        "gn_scale": np.ones(C, np.float32),
        "gn_bias": np.zeros(C, np.float32),
    }
    for w in ("wq", "wk", "wv", "wp"):
        ins[w] = (rng.standard_normal((C, C)) / 16.0).astype(np.float32)
    for b in ("bq", "bk", "bv", "bp"):
        ins[b] = np.zeros(C, np.float32)
    out = kernel(**ins)
    print(out.shape, out.dtype, float(np.abs(out).mean()))


# revision 35
# speedup vs baseline: 1.3165x; 1.3165x over previous
"""AttentionBlock (GroupNorm + single-head self-attention + proj + residual)
for Trainium2, 8 NeuronCores, data-parallel over (batch, token-half).

Shapes (hardcoded): x [4, 256, 64, 64] fp32, weights [256, 256] fp32.
Each core handles one (batch b, token-half h): 2048 query tokens against the
full 4096 keys/values of its batch, entirely in SBUF.

v2: fp8(e4m3) DoubleRow matmuls for QKV production, scores and PV
(K=256 contraction in one instruction at 0.5 cyc/row); GroupNorm applied as
an explicit normalized image x8 = seff*x + beff cast to fp8, so the conv
weights are host-precast fp8 constants (scaled by 8 to stay in fp8's normal
range; compensated exactly by the exp argument scale and a 1/8 output
scale). Scores are exp'd in [128,1024] two-bank PSUM tiles; exp runs on ACT
(exact spline) for most tiles and optionally on DVE via a Schraudolph
int16-bitcast-fp16 + fp16->fp8 convert for load balancing. The softmax
denominator comes from a ones-column appended to V; normalization, the
attn transpose (PE, fp16), fp16 projection, +bias (ACT, with the 1/8
compensation), residual add and DMA-out complete the pipeline.

Accuracy: fp8 quantization of x8/w/k/q/u/v perturbs softmax weights by a few
percent, but the output is residual-dominated (||proj||/||x|| ~ 2.6%), so
end-to-end relative error stays ~1-3e-3, well under the 2e-2 gate.
"""

import sys

try:
    import concourse.bass as bass  # noqa: F401
except ImportError:
    sys.path.insert(0, "/opt/trn_rl_repo")

import numpy as np
import ml_dtypes

import concourse.bass as bass
import concourse.mybir as mybir
import concourse.tile as tile
from concourse.bass import ts
from concourse.bass_utils import run_bass_kernel_spmd
from concourse.masks import make_identity

FP = mybir.dt.float32
F16 = mybir.dt.float16
F8 = mybir.dt.float8e4
I16 = mybir.dt.int16
AF = mybir.ActivationFunctionType
ALU = mybir.AluOpType
AX = mybir.AxisListType
DR = mybir.MatmulPerfMode.DoubleRow

P = 128
C = 256
HW = 4096
HALF = 2048
NCH = 2          # channel chunks of 128
NJT = 32         # 128-wide key tiles
NJP = 16         # key-tile pairs (256 keys each)
NIG = 4          # query i-groups of 512
NCHUNK = 8       # 512-wide token chunks of the full image
GROUPS = 32
GSIZE = C // GROUPS          # 8 channels per group
NELEM = GSIZE * HW           # 32768 elements per group
EPS = 1e-6
WS = 8.0                     # host weight prescale (wq,wk,wv *= 8)
SCALE = float(C) ** -0.5     # 0.0625 softmax scale
EXPB = -3.0                  # global exp bias: u = exp(s*SCALE + EXPB); real score tails reach ~7.5 sigma, keep exp(max)+margin under fp8's 240
# PSUM holds 64*s (8x on q and k), so the activation scale is SCALE/64
PSC = SCALE / (WS * WS)
# Schraudolph fp8 constants: bits8 = rne(a8*s_psum + b8) as saturating uint8
# (negatives flush to 0 = zero weight), bitcast to fp8e4. DVE-side exp.
A8 = 8.0 / np.log(2.0) * PSC
B8 = 8.0 * (7.0 + EXPB / np.log(2.0)) - 0.5


def _split_waits(nc, max_waits=1):
    """The pinned walrus rejects >1 sync-wait on ctrl instructions; hoist
    excess waits onto preceding NoOps on the same engine (same instruction
    stream, so ordering is preserved)."""
    ctr = 0
    for bb in nc.m.functions[0].blocks:
        out = []
        changed = False
        for inst in bb.instructions:
            si = getattr(inst, "sync_info", None)
            waits = list(si.on_wait) if (si and si.on_wait) else []
            if len(waits) > max_waits:
                changed = True
                head, rest = waits[:-max_waits], waits[-max_waits:]
                for k in range(0, len(head), max_waits):
                    ctr += 1
                    nop = mybir.InstNoOp(name=f"I-wsplit-{ctr}", ins=[], outs=[])
                    nop.engine = inst.engine
                    nop.sync_info = mybir.SyncInfo(
                        on_wait=head[k : k + max_waits], on_update=[]
                    )
                    out.append(nop)
                inst.sync_info = mybir.SyncInfo(
                    on_wait=rest, on_update=list(si.on_update or [])
                )
            out.append(inst)
        if changed:
            bb.instructions = out


def build_nc(split=True, reps=1, n_dve_exp=28, gp_resid=True,
             dr_s=True, dr_pv=True, dr_prod=True, tps_acc=False, gp_conv=False,
             gp_xhat=True, drain=False, qreorder=True, dbg=False, att_dve=False,
             v_lazy=True, q_lazy=True):
    """n_dve_exp: how many of the 64 (g,jp) exp tiles run on DVE
    (Schraudolph) instead of ACT. gp_resid: residual add on GPSIMD."""
    nc = bass.Bass()
    xf = nc.dram_tensor("xf", [C, HW], F16, kind="ExternalInput")
    w8q = nc.dram_tensor("w8q", [P, NCH, C], F8, kind="ExternalInput")
    w8k = nc.dram_tensor("w8k", [P, NCH, C], F8, kind="ExternalInput")
    w8v = nc.dram_tensor("w8v", [P, NCH, C], F8, kind="ExternalInput")
    wp16 = nc.dram_tensor("wp16", [P, NCH, C], F16, kind="ExternalInput")
    bq8 = nc.dram_tensor("bq8", [P, NCH], FP, kind="ExternalInput")
    bp2 = nc.dram_tensor("bp2", [P, NCH], FP, kind="ExternalInput")
    gns = nc.dram_tensor("gns", [P, NCH], FP, kind="ExternalInput")
    gnb = nc.dram_tensor("gnb", [P, NCH], FP, kind="ExternalInput")
    gsel = nc.dram_tensor("gsel", [P, P], FP, kind="ExternalInput")
    y = nc.dram_tensor("y", [C, HALF], FP, kind="ExternalOutput")
    if dbg:
        dbg_t = {
            "d_seff": nc.dram_tensor("d_seff", [P, NCH], FP, kind="ExternalOutput"),
            "d_beff": nc.dram_tensor("d_beff", [P, NCH], FP, kind="ExternalOutput"),
            "d_qb2": nc.dram_tensor("d_qb2", [P, NCH], FP, kind="ExternalOutput"),
            "d_ob2": nc.dram_tensor("d_ob2", [P, NCH], FP, kind="ExternalOutput"),
            "d_k8": nc.dram_tensor("d_k8", [P, 512], FP, kind="ExternalOutput"),
            "d_q8": nc.dram_tensor("d_q8", [P, 512], FP, kind="ExternalOutput"),
            "d_v8": nc.dram_tensor("d_v8", [P, 512], FP, kind="ExternalOutput"),
            "d_uA": nc.dram_tensor("d_uA", [P, 1024], FP, kind="ExternalOutput"),
            "d_uD": nc.dram_tensor("d_uD", [P, 1024], FP, kind="ExternalOutput"),
            "d_acc": nc.dram_tensor("d_acc", [P, C + 2], FP, kind="ExternalOutput"),
        }

    # static exp-engine schedule: spread DVE tiles evenly over the 64 slots
    dve_slot = [False] * (NIG * NJP)
    if n_dve_exp > 0:
        step = (NIG * NJP) / float(n_dve_exp)
        for i in range(n_dve_exp):
            dve_slot[min(63, int(i * step + step / 2))] = True

    with tile.TileContext(nc) as tc:
        with (
            tc.tile_pool(name="wts", bufs=1) as wts,
            tc.tile_pool(name="big", bufs=1) as big,
            tc.tile_pool(name="upool", bufs=6) as upool,
            tc.tile_pool(name="small", bufs=10) as small,
            tc.tile_pool(name="stats", bufs=1) as stats,
            tc.tile_pool(name="outp", bufs=3) as outp,
            tc.tile_pool(name="psS", bufs=2, space="PSUM") as psS,
            tc.tile_pool(name="psAcc", bufs=4, space="PSUM") as psAcc,
        ):
            # ---------------- input image first (critical path) ----------------
            # few, large DMAs: per-dma_start queue overhead dominates small
            # transfers; o=0 on the sync queue, o=1 on the scalar queue
            xf_sb = big.tile([P, NCH, HW], F16, tag="xf")
            dma_engines = [nc.sync, nc.scalar]
            for o in range(NCH):
                for q in range(4):
                    dma_engines[o].dma_start(
                        out=xf_sb[:, o, ts(q, 1024)],
                        in_=xf[o * P : (o + 1) * P, ts(q, 1024)],
                    )

            # ---------------- constants ----------------
            w_sb = {}
            for name, dram, dt_ in (
                ("w8q", w8q, F8), ("w8k", w8k, F8), ("w8v", w8v, F8),
                ("wp16", wp16, F16),
            ):
                t = wts.tile([P, NCH, C], dt_, tag=f"w_{name}")
                nc.sync.dma_start(out=t, in_=dram[:, :, :])
                w_sb[name] = t
            gsel_sb = wts.tile([P, P], FP, tag="gsel")
            nc.sync.dma_start(out=gsel_sb, in_=gsel[:, :])
            bq_sb = wts.tile([P, NCH], FP, tag="bq")
            nc.sync.dma_start(out=bq_sb, in_=bq8[:, :])
            bp_sb = wts.tile([P, NCH], FP, tag="bp")
            nc.sync.dma_start(out=bp_sb, in_=bp2[:, :])
            gns_sb = wts.tile([P, NCH], FP, tag="gns")
            nc.sync.dma_start(out=gns_sb, in_=gns[:, :])
            gnb_sb = wts.tile([P, NCH], FP, tag="gnb")
            nc.sync.dma_start(out=gnb_sb, in_=gnb[:, :])
            ident_fp = wts.tile([P, P], FP, tag="ident_fp")
            make_identity(nc, ident_fp)
            ident = wts.tile([P, P], F16, tag="ident")
            nc.vector.tensor_copy(ident, ident_fp)
            expb_sb = wts.tile([P, 1], FP, tag="expb")
            nc.vector.memset(expb_sb, EXPB)

            for _rep in range(reps):
                # ---------------- phase 1a: GroupNorm statistics ----------------
                # bn_stats gives (count, mean, M2) for even/odd halves per chunk
                bnout = stats.tile([P, NCH, NCHUNK, 2, 3], FP, tag="bno")
                for t8 in range(NCHUNK):
                    for o in range(NCH):
                        nc.vector.bn_stats(
                            out=bnout[:, o, t8], in_=xf_sb[:, o, ts(t8, 512)]
                        )

                seff = stats.tile([P, NCH], FP, tag="seff")
                beff = stats.tile([P, NCH], FP, tag="beff")
                eps_sb = stats.tile([P, 1], FP, tag="eps")
                nc.vector.memset(eps_sb, EPS)
                for o in range(NCH):
                    part = stats.tile([P, 2], FP, tag=f"part{o}")
                    means = bnout[:, o, :, :, 1]
                    m2s = bnout[:, o, :, :, 2]
                    # part0 = sum(means) = chunk_sums/256 ; part1 = sumsq
                    nc.vector.tensor_reduce(
                        out=part[:, 0:1], in_=means, axis=AX.XY, op=ALU.add
                    )
                    m2red = stats.tile([P, 1], FP, tag=f"m2red{o}")
                    nc.vector.tensor_reduce(
                        out=m2red, in_=m2s, axis=AX.XY, op=ALU.add
                    )
                    mscr = stats.tile([P, NCHUNK, 2], FP, tag=f"mscr{o}")
                    mss = stats.tile([P, 1], FP, tag=f"mss{o}")
                    nc.vector.tensor_tensor(mscr, means, means, ALU.mult)
                    nc.vector.tensor_reduce(
                        out=mss, in_=mscr, axis=AX.XY, op=ALU.add
                    )
                    nc.vector.tensor_scalar_mul(mss, mss, 256.0)
                    nc.vector.tensor_tensor(part[:, 1:2], m2red, mss, ALU.add)
                    gps = psAcc.tile([P, 258], FP, tag="acc", name="gps")[:, :2]
                    nc.tensor.matmul(gps, lhsT=gsel_sb, rhs=part, start=True, stop=True)
                    mean = stats.tile([P, 1], FP, tag=f"mean{o}")
                    nc.vector.tensor_scalar_mul(mean, gps[:, 0:1], 256.0 / NELEM)
                    ex2 = stats.tile([P, 1], FP, tag=f"ex2{o}")
                    nc.vector.tensor_scalar_mul(ex2, gps[:, 1:2], 1.0 / NELEM)
                    msq = stats.tile([P, 1], FP, tag=f"msq{o}")
                    nc.vector.tensor_mul(msq, mean, mean)
                    var = stats.tile([P, 1], FP, tag=f"var{o}")
                    nc.vector.tensor_tensor(var, ex2, msq, ALU.subtract)
                    # rstd = exp(-0.5 * ln(var + eps)) — stays in the exp table set
                    lnv = stats.tile([P, 1], FP, tag=f"lnv{o}")
                    nc.scalar.activation(out=lnv, in_=var, func=AF.Ln, bias=eps_sb)
                    rstd = stats.tile([P, 1], FP, tag=f"rstd{o}")
                    nc.scalar.activation(out=rstd, in_=lnv, func=AF.Exp, scale=-0.5)
                    nc.vector.tensor_mul(seff[:, o : o + 1], gns_sb[:, o : o + 1], rstd)
                    tmp = stats.tile([P, 1], FP, tag=f"tmp{o}")
                    nc.vector.tensor_mul(tmp, mean, seff[:, o : o + 1])
                    nc.vector.tensor_tensor(
                        beff[:, o : o + 1], gnb_sb[:, o : o + 1], tmp, ALU.subtract
                    )

                # ---------------- phase 1b: fold GN scale into fp16 weights ----------------
                # w2x = w8x * seff (per input channel); production runs fp16
                # on the raw fp16 image, skipping an explicit normalized-image
                # pass. The beff (GN shift) terms: K's drops out of softmax
                # exactly; Q's becomes qb2 (matvec); V's flows through softmax
                # as a per-channel constant, folded into the output bias ob2
                # via wp (matvec). All matvecs are tiny [P,1] matmuls.
                w2 = {}
                for wname in ("w8q", "w8k", "w8v"):
                    w2[wname] = wts.tile([P, NCH, C], F16, tag=f"w2_{wname}", name=f"w2{wname}")
                    for ci in range(NCH):
                        nc.vector.tensor_scalar_mul(
                            w2[wname][:, ci], w_sb[wname][:, ci], seff[:, ci : ci + 1]
                        )
                beff8 = stats.tile([P, NCH], F8, tag="beff8")
                nc.vector.tensor_copy(beff8, beff)
                qb2 = stats.tile([P, NCH], FP, tag="qb2")
                ob2 = stats.tile([P, NCH], FP, tag="ob2")
                vc16 = stats.tile([P, NCH], F16, tag="vc16")
                for oo in range(NCH):
                    mv = psAcc.tile([P, 512], FP, tag="acc", name="mvq")[:, :1]
                    for ci in range(NCH):
                        nc.tensor.matmul(
                            mv, lhsT=w_sb["w8q"][:, ci, oo * P : (oo + 1) * P],
                            rhs=beff8[:, ci : ci + 1],
                            start=(ci == 0), stop=(ci == NCH - 1),
                        )
                    nc.vector.tensor_tensor(
                        qb2[:, oo : oo + 1], mv, bq_sb[:, oo : oo + 1], ALU.add
                    )
                    mv2 = psAcc.tile([P, 512], FP, tag="acc", name="mvv")[:, :1]
                    for ci in range(NCH):
                        nc.tensor.matmul(
                            mv2, lhsT=w_sb["w8v"][:, ci, oo * P : (oo + 1) * P],
                            rhs=beff8[:, ci : ci + 1],
                            start=(ci == 0), stop=(ci == NCH - 1),
                        )
                    nc.vector.tensor_copy(vc16[:, oo : oo + 1], mv2)
                for oo in range(NCH):
                    mv3 = psAcc.tile([P, 512], FP, tag="acc", name="mvp")[:, :1]
                    for ci in range(NCH):
                        nc.tensor.matmul(
                            mv3, lhsT=w_sb["wp16"][:, ci, oo * P : (oo + 1) * P],
                            rhs=vc16[:, ci : ci + 1],
                            start=(ci == 0), stop=(ci == NCH - 1),
                        )
                    nc.vector.tensor_scalar(
                        out=ob2[:, oo : oo + 1], in0=mv3,
                        scalar1=1.0 / WS, scalar2=bp_sb[:, oo : oo + 1],
                        op0=ALU.mult, op1=ALU.add,
                    )

                # ---------------- phase 1c: K, V, Q (fp16 production) ----------------
                k8 = big.tile([P, NCH, HW], F8, tag="k8")
                v8 = big.tile([P, NJT, C + 16], F8, tag="v8")  # 272-stride: DR subtile step must be 16B-aligned
                nc.vector.memset(v8[:, :, C : C + 16], 0.0)
                nc.vector.memset(v8[:, :, C : C + 1], 1.0)
                # K: pairs of 512-token chunks -> one 2-bank PSUM -> one copy
                for oo in range(NCH):
                    for t4 in range(4):
                        ps = psS.tile([P, 1024], FP, tag="psS", name="psk")
                        for h in range(2):
                            for ci in range(NCH):
                                nc.tensor.matmul(
                                    ps[:, h * 512 : (h + 1) * 512],
                                    lhsT=w2["w8k"][:, ci, oo * P : (oo + 1) * P],
                                    rhs=xf_sb[:, ci, ts(2 * t4 + h, 512)],
                                    start=(ci == 0), stop=(ci == NCH - 1),
                                )
                        nc.scalar.copy(
                            out=k8[:, oo, ts(t4, 1024)], in_=ps
                        )
                # V: produced lazily (interleaved into the g=0 attention loop)
                def produce_v(q4):
                    ps = psS.tile([P, 4, 256], FP, tag="psS", name="psv")
                    for h in range(4):
                        j = q4 * 4 + h
                        for ci in range(NCH):
                            nc.tensor.matmul(
                                ps[:, h],
                                lhsT=xf_sb[:, ci, j * P : (j + 1) * P],
                                rhs=w2["w8v"][:, ci],
                                start=(ci == 0), stop=(ci == NCH - 1),
                            )
                    if q4 % 2 == 0:
                        nc.scalar.copy(out=v8[:, ts(q4, 4), 0:C], in_=ps)
                    else:
                        nc.vector.tensor_copy(out=v8[:, ts(q4, 4), 0:C], in_=ps)
                # Q: pairs of 512-query chunks, bias fused into evacuation
                q8t = big.tile([P, NCH, HALF], F8, tag="q8")

                def produce_q(t2):
                    for oo in range(NCH):
                        ps = psS.tile([P, 1024], FP, tag="psS", name="psq")
                        for h in range(2):
                            for ci in range(NCH):
                                nc.tensor.matmul(
                                    ps[:, h * 512 : (h + 1) * 512],
                                    lhsT=w2["w8q"][:, ci, oo * P : (oo + 1) * P],
                                    rhs=xf_sb[:, ci, ts(2 * t2 + h, 512)],
                                    start=(ci == 0), stop=(ci == NCH - 1),
                                )
                        nc.vector.tensor_scalar_add(
                            q8t[:, oo, ts(t2, 1024)], ps, qb2[:, oo : oo + 1]
                        )

                produce_q(0)
                if not q_lazy:
                    produce_q(1)

                if dbg:
                    def dump(dst, src_ap, n):
                        scr = outp.tile([P, n], FP, tag="dbgscr", name="dbgscr")
                        nc.vector.tensor_copy(scr, src_ap)
                        nc.sync.dma_start(out=dst[:, :], in_=scr)
                    dump(dbg_t["d_seff"], seff, NCH)
                    dump(dbg_t["d_beff"], beff, NCH)
                    dump(dbg_t["d_qb2"], qb2, NCH)
                    dump(dbg_t["d_ob2"], ob2, NCH)
                    dump(dbg_t["d_k8"], k8[:, 0, 0:512], 512)
                    dump(dbg_t["d_q8"], q8t[:, 0, 0:512], 512)

                # ---------------- phase 2: attention ----------------
                # Cross-group software pipeline: the first two (S,exp) tiles
                # of group g+1 are emitted BEFORE group g's evacuation
                # (normalize/transpose/proj/out), so ACT keeps streaming exp
                # while PE drains the evacuation. tps/proj PSUM come from the
                # psAcc ring (transiently free between acc generations); psS
                # is exclusively the S/production ring.
                att16 = big.tile([P, NCH, HALF], F16, tag="att")
                slot = 0

                def s_exp(g, jp):
                    nonlocal slot
                    ps = psS.tile([P, 2, 512], FP, tag="psS", name="pss")
                    for h in range(2):
                        j = 2 * jp + h
                        if dr_s:
                            nc.tensor.matmul(
                                ps[:, h],
                                lhsT=k8[:, :, j * P : (j + 1) * P],
                                rhs=q8t[:, :, ts(g, 512)],
                                start=True, stop=True, perf_mode=DR,
                            )
                        else:
                            for ci in range(NCH):
                                nc.tensor.matmul(
                                    ps[:, h],
                                    lhsT=k8[:, ci, j * P : (j + 1) * P],
                                    rhs=q8t[:, ci, ts(g, 512)],
                                    start=(ci == 0), stop=(ci == NCH - 1),
                                )
                    if dve_slot[slot]:
                        # one-op Schraudolph exp on DVE: affine into saturating
                        # uint8 whose bits are the fp8e4 encoding of exp()
                        u8i = upool.tile([P, 2, 512], mybir.dt.uint8, tag="u")
                        nc.vector.tensor_scalar(
                            out=u8i, in0=ps, scalar1=float(A8),
                            scalar2=float(B8), op0=ALU.mult, op1=ALU.add,
                        )
                        u2 = u8i.bitcast(F8)
                    else:
                        u2 = upool.tile([P, 2, 512], F8, tag="u")
                        nc.scalar.activation(
                            out=u2, in_=ps, func=AF.Exp, scale=PSC,
                            bias=expb_sb,
                        )
                    if dbg and slot == 0:
                        du = outp.tile([P, 1024], FP, tag="dbgu", name="dbguA")
                        nc.vector.tensor_copy(du, u2)
                        nc.sync.dma_start(out=dbg_t["d_uA"][:, :], in_=du)
                    if dbg and dve_slot[slot] and slot == min(
                        i for i, v in enumerate(dve_slot) if v
                    ):
                        du = outp.tile([P, 1024], FP, tag="dbgu", name="dbguD")
                        nc.vector.tensor_copy(du, u2)
                        nc.sync.dma_start(out=dbg_t["d_uD"][:, :], in_=du)
                    slot += 1
                    return u2

                def pv(acc, jp, u2):
                    for t in range(4):
                        if dr_pv:
                            nc.tensor.matmul(
                                acc[t],
                                lhsT=u2[:, :, t * P : (t + 1) * P],
                                rhs=v8[:, 2 * jp : 2 * jp + 2, 0 : C + 2],
                                start=(jp == 0), stop=(jp == NJP - 1),
                                perf_mode=DR,
                            )
                        else:
                            for h in range(2):
                                nc.tensor.matmul(
                                    acc[t],
                                    lhsT=u2[:, h, t * P : (t + 1) * P],
                                    rhs=v8[:, 2 * jp + h, 0 : C + 2],
                                    start=(jp == 0 and h == 0),
                                    stop=(jp == NJP - 1 and h == 1),
                                )

                def evac(pg, pasbs, drain=False):
                    for t in range(4):
                        tps = psAcc.tile([P, 2, P], F16, tag="acc", name="tps")
                        for o in range(NCH):
                            nc.tensor.transpose(
                                tps[:, o], pasbs[t][:, o * P : (o + 1) * P], ident
                            )
                        col = pg * 512 + t * P
                        if att_dve:
                            nc.vector.tensor_copy(att16[:, :, col : col + P], tps)
                        else:
                            nc.scalar.copy(out=att16[:, :, col : col + P], in_=tps)
                        if drain:
                            # last group: project/emit per 128-col slice so the
                            # tail pipelines instead of serializing after PE
                            _proj_out(pg, col, P)
                    if not drain:
                        _proj_out(pg, pg * 512, 512)

                def _proj_out(pg, col, width):
                    for oo in range(NCH):
                        ps = psAcc.tile([P, 512], FP, tag="acc", name="psp")[:, :width]
                        for oi in range(NCH):
                            nc.tensor.matmul(
                                ps,
                                lhsT=w_sb["wp16"][:, oi, oo * P : (oo + 1) * P],
                                rhs=att16[:, oi, col : col + width],
                                start=(oi == 0), stop=(oi == NCH - 1),
                            )
                        ot = outp.tile([P, 512], FP, tag="out", name="ot")[:, :width]
                        nc.scalar.activation(
                            out=ot, in_=ps, func=AF.Identity,
                            bias=ob2[:, oo : oo + 1], scale=1.0 / WS,
                        )
                        gcol = col - pg * 512
                        if gp_resid and not width < 512:
                            nc.gpsimd.tensor_tensor(
                                ot, ot, xf_sb[:, oo, pg * 512 + gcol : pg * 512 + gcol + width], ALU.add
                            )
                        else:
                            nc.vector.tensor_add(
                                ot, ot, xf_sb[:, oo, pg * 512 + gcol : pg * 512 + gcol + width]
                            )
                        nc.sync.dma_start(
                            out=y[oo * P : (oo + 1) * P, col : col + width],
                            in_=ot,
                        )

                pre = []        # [(jp, u2)] prefetched for the current group
                for g in range(NIG):
                    acc = [
                        psAcc.tile([P, C + 2], FP, tag="acc", name=f"acc{g}_{t}")
                        for t in range(4)
                    ]
                    first = len(pre)
                    if g == 0:
                        produce_v(0)
                        if not v_lazy:
                            for q4 in range(1, NCHUNK):
                                produce_v(q4)
                    # issue S/exp one jp ahead of PV so the PE stream stages
                    # the next S-tile before each PV burst
                    if first < NJP:
                        pre.append((first, s_exp(g, first)))
                    for jp, u2 in pre[:-1] if first < NJP else pre:
                        pv(acc, jp, u2)
                    lead = pre[-1] if first < NJP else None
                    pre = []
                    for jp in range(first, NJP):
                        if v_lazy and g == 0 and jp % 2 == 0 and jp // 2 + 1 < NCHUNK:
                            produce_v(jp // 2 + 1)
                        if q_lazy and g == 0 and jp == 13:
                            produce_q(1)
                        nxt = s_exp(g, jp + 1) if jp + 1 < NJP else None
                        pv(acc, jp, lead[1])
                        lead = (jp + 1, nxt)
                    # normalize (frees the acc ring), then prefetch next
                    # group's first S/exp tiles, then evacuate this group
                    if dbg and g == 0:
                        dv = outp.tile([P, 512], FP, tag="dbgscr", name="dbgv8")
                        nc.vector.tensor_copy(dv, v8[:, 0:2, 0:256])
                        nc.sync.dma_start(out=dbg_t["d_v8"][:, :], in_=dv)
                        dacc = outp.tile([P, C + 2], FP, tag="dbgacc", name="dbgacc")
                        nc.vector.tensor_copy(dacc, acc[0])
                        nc.sync.dma_start(out=dbg_t["d_acc"][:, :], in_=dacc)
                    asbs = []
                    for t in range(4):
                        rs = small.tile([P, 1], FP, tag="rs")
                        nc.vector.reciprocal(rs, acc[t][:, C : C + 1])
                        asb = small.tile([P, C], F16, tag="asb", name=f"asb{g}_{t}")
                        nc.vector.tensor_scalar_mul(asb, acc[t][:, 0:C], rs)
                        asbs.append(asb)
                    if g + 1 < NIG:
                        pre = [(0, s_exp(g + 1, 0)), (1, s_exp(g + 1, 1))]
                    evac(g, asbs, drain=(drain and g == NIG - 1))

    if split:
        _split_waits(nc)
    return nc


_NC_CACHE = None


def _get_nc():
    global _NC_CACHE
    if _NC_CACHE is None:
        _NC_CACHE = build_nc()
    return _NC_CACHE


def make_in_maps(x, gn_scale, gn_bias, wq, bq, wk, bk, wv, bv, wp, bp):
    B = x.shape[0]
    f32 = np.float32
    f8 = ml_dtypes.float8_e4m3

    def col2(v):  # [256] -> [128, 2], column o = channels o*128..o*128+127
        return np.ascontiguousarray(np.asarray(v, f32).reshape(NCH, P).T)

    def w3d(w, dt, scale=1.0):
        # [C(out), C(in)] -> [P, NCH(ci), C(out)]: [p, ci, co] = w[co, ci*P+p]
        wt = np.asarray(w, f32).T * scale            # [ci_full, co]
        wt = np.clip(wt, -240.0, 240.0)
        return np.ascontiguousarray(
            wt.reshape(NCH, P, C).transpose(1, 0, 2)
        ).astype(dt)

    w8q_h = w3d(wq, f8, WS)
    w8k_h = w3d(wk, f8, WS)
    w8v_h = w3d(wv, f8, WS)
    wp16_h = w3d(wp, np.float16, 1.0)
    bp2 = col2(np.asarray(wp, f32) @ np.asarray(bv, f32) + np.asarray(bp, f32))
    bq8 = col2(np.asarray(bq, f32) * WS)
    gns = col2(gn_scale)
    gnb = col2(gn_bias)
    gsel = np.kron(np.eye(P // GSIZE, dtype=f32), np.ones((GSIZE, GSIZE), f32))

    xr = np.asarray(x, f32).reshape(B, C, HW)
    in_maps = []
    for core in range(8):
        b, h = core // 2, core % 2
        # rotate so this core's query half sits at columns 0..HALF-1
        # (attention/groupnorm are token-order invariant)
        xfb = np.ascontiguousarray(np.roll(xr[b], -h * HALF, axis=1))
        in_maps.append(
            {
                "xf": xfb.astype(np.float16),
                "w8q": w8q_h, "w8k": w8k_h, "w8v": w8v_h, "wp16": wp16_h,
                "bq8": bq8, "bp2": bp2, "gns": gns, "gnb": gnb, "gsel": gsel,
            }
        )
    return in_maps


def assemble_out(results, B=4):
    out = np.empty((B, C, HW), np.float32)
    for core in range(8):
        b, h = core // 2, core % 2
        out[b, :, h * HALF : (h + 1) * HALF] = results[core]["y"]
    return out.reshape(B, C, 64, 64)


def kernel(**inputs):
    in_maps = make_in_maps(**inputs)
    nc = _get_nc()
    res = run_bass_kernel_spmd(nc, in_maps, list(range(8)))
    return assemble_out(res.results, B=inputs["x"].shape[0])


if __name__ == "__main__":
    rng = np.random.default_rng(0)
    ins = {
        "x": rng.standard_normal((4, C, 64, 64)).astype(np.float32),
        "gn_scale": np.ones(C, np.float32),
        "gn_bias": np.zeros(C, np.float32),
    }
    for w in ("wq", "wk", "wv", "wp"):
        ins[w] = (rng.standard_normal((C, C)) / 16.0).astype(np.float32)
    for b in ("bq", "bk", "bv", "bp"):
        ins[b] = np.zeros(C, np.float32)
    out = kernel(**ins)
    print(out.shape, out.dtype, float(np.abs(out).mean()))
